# revision 29
# baseline (speedup 1.0000x reference)
"""Trainium2 Bass kernel for nn_BlankEmbedding (embedding gather + blank-run scan).

Math: the reference computes e = emb_table[x], then runs 8 iterations of
    pos = shift_right(pos); acc = shift_right(acc); out = out + acc; acc = out*pos
starting from pos = is_preblank.  Unrolling, out[i] = sum_{d=0..8} C[i,d]*e[i-d]
with banded integer coefficients C depending only on x; C[i,0] == 1 and rows
with any C[i,d>0] != 0 are rare (~1/128 blank density -> ~136 rows per core).

Device strategy (per core, 2048 of the 16384 rows, data-parallel over B*S):
  * the deduplicated table is converted to bf16 on the host; the device works
    in bf16 end-to-end (gather, writeback, matmul, scatter-add) and the host
    upcasts the result to fp32.  bf16 quantization is ~2^-9 relative error,
    far under the 2e-2 gate, and halves every DMA byte.
  * main path: dma_gather the core's 2048 rows (5 SWDGE chunks into a fully
    resident SBUF buffer - 64KB/partition in bf16, so gathers never wait on
    writebacks).  The SBUF row layout is chosen per *writeback group* (rows
    0-1024 / 1024-1920 / 1920-2048) so the output is written with just three
    large strided HWDGE DMAs - each HWDGE ring cycle costs ~6us of issue +
    completion latency on top of the transfer, so fewer, bigger writebacks
    win.  Gathers only wait on the index DMA, not on W/cnts.
  * affected rows (grouped <=128 per output half): their band rows e[i-d] are
    already in the main SBUF buffer, so the correction
        delta[r, :] = sum_d C[r,d] * e[r-d]
    is computed on the (otherwise idle) tensor engine as
        delta = sum_k W_k[p, r] . mbuf[p, k, :]
    with a host-built sparse bf16 coefficient matrix W (one [128,128] chunk
    per mbuf free slot k), accumulated in fp32 PSUM.  Band rows that fall
    before the core's row range (cross-half) come from one small padded
    dma_gather (xbuf) with its own W chunk.  The PSUM delta is copied to
    bf16 by DVE and dma_scatter_add-ed onto the output rows as soon as the
    covering writebacks land.  No per-depth gathers, no DVE chains.

Host side computes index lists / coefficient matrices from x ([B,S] int
ops), the bf16 table conversion, and reassembles + upcasts the outputs.
"""

import numpy as np
import ml_dtypes

BF16 = ml_dtypes.bfloat16

B, S, D = 4, 4096, 2048
N_CORES = 8
RPC = (B * S) // N_CORES          # rows per core = 2048
# writeback groups: (row start, row end, rows per partition, first mbuf slot)
# group 0 covers the first output half exactly (gates the first scatter_add);
# the small tail group keeps the final writeback (which gates the last
# scatter_add) short
WBG = [(0, 1024, 8, 0), (1024, 2048, 8, 8)]
N_WB = len(WBG)
# gather chunks: (wb group, first in-partition slot, n slots)
CHUNKS = [(0, 0, 4), (0, 4, 4), (1, 0, 4), (1, 4, 4)]
N_CHUNKS = len(CHUNKS)
CHUNK_SIZES = [ns * 128 for (_, _, ns) in CHUNKS]
CPCS = [cs // 16 for cs in CHUNK_SIZES]   # idx columns per chunk
CPC_OFF = [sum(CPCS[:i]) for i in range(N_CHUNKS + 1)]
N_SLOTS = sum(ns for (_, _, ns) in CHUNKS)
# mbuf slot -> gather chunk (for tensor-engine waits)
SLOT_CHUNK = {}
for _ci, (_wg, _gb, _ns) in enumerate(CHUNKS):
    for _g in range(_ns):
        SLOT_CHUNK[WBG[_wg][3] + _gb + _g] = _ci
# gather chunks covering each output half (for scatter gating)
WB_OF_HALF = [[0, 1], [2, 3]]
NBLANK_IDS = 16
N_ITER = 8
BAND = N_ITER + 1                 # out[i] depends on e[i-8..i]
FSTEP = 512                       # one PSUM bank of fp32 per matmul


def _cdiv(a, b):
    return (a + b - 1) // b


def _row_to_pk(l):
    """mbuf location (partition, free slot) of local row l (matches midx)."""
    for (st, en, gpw, s0) in WBG:
        if st <= l < en:
            w = l - st
            return w // gpw, s0 + w % gpw
    raise ValueError(l)


def _chunk_rows(ci):
    """local row index held by gather slot j of chunk ci, for all j."""
    wg, gb, ns = CHUNKS[ci]
    st, _, gpw, _ = WBG[wg]
    j = np.arange(ns * 128)
    return st + (j % 128) * gpw + gb + j // 128


def _compute_coeffs(x):
    """C[b, s, d] for d=0..8 (float64 holds small ints exactly), per batch row."""
    b, s = x.shape
    blank = ((x >= 0) & (x < NBLANK_IDS)).astype(np.float64)
    shift_r = lambda t: np.concatenate([np.zeros_like(t[:, :1]), t[:, :-1]], axis=1)
    first = np.maximum(blank - shift_r(blank), 0.0)
    m = np.concatenate([first[:, 1:], np.zeros_like(first[:, :1])], axis=1)  # preblank
    C = np.zeros((b, s, BAND))
    C[:, :, 0] = 1.0
    for k in range(1, N_ITER + 1):
        m_k = np.zeros_like(m)
        m_k[:, k:] = m[:, :-k]                       # m[i-k]
        Cs = np.zeros_like(C)
        Cs[:, 1:, 1:] = C[:, :-1, :-1]               # C[i-1, d-1]
        C = C + m_k[:, :, None] * Cs
    return C


def _wrap16(vals, ncols):
    """Wrap a 1-D index list into the [128, ncols] int16 layout the SWDGE
    gather/scatter ucode expects: slot j at [j % 16, j // 16], and the 16-row
    block replicated across all eight 16-partition Q7 core groups."""
    blk = np.zeros((16, ncols), dtype=np.int16)
    v = np.asarray(vals, dtype=np.int16)
    blk[np.arange(len(v)) % 16, np.arange(len(v)) // 16] = v
    return np.tile(blk, (8, 1))


def _prepare(x_np):
    """All host-side index/coefficient prep. Returns per-core arrays + meta."""
    uniq, inv = np.unique(x_np, return_inverse=True)
    ridx = inv.reshape(x_np.shape).astype(np.int64)   # x remapped to table rows
    NV = len(uniq)
    assert NV <= 32767, "int16 gather index overflow"

    C = _compute_coeffs(x_np)
    assert (np.abs(C) <= 256).all(), "coefficients exceed bf16-exact range"
    aff = (C[:, :, 1:] != 0).any(axis=2)              # [B,S]

    cores = []
    for c in range(N_CORES):
        b, h = c // 2, c % 2
        s0 = h * RPC
        midx = np.zeros((128, CPC_OFF[-1]), dtype=np.int16)
        for ci in range(N_CHUNKS):
            rows = _chunk_rows(ci)
            midx[:, CPC_OFF[ci]:CPC_OFF[ci + 1]] = _wrap16(
                ridx[b, s0 + rows], CPCS[ci])

        # affected rows split by output half: the group over rows < RPC/2 can
        # scatter as soon as the first writeback group lands
        rows_all = np.nonzero(aff[b, s0:s0 + RPC])[0]
        Cc = C[b, s0:s0 + RPC]                        # [RPC, 9] (local view)
        halves = [rows_all[(rows_all >= hh * (RPC // 2))
                           & (rows_all < (hh + 1) * (RPC // 2))]
                  for hh in range(2)]
        cores.append(dict(b=b, s0=s0, halves=halves, Cc=Cc, midx=midx))

    H = [max(_cdiv(len(co["halves"][h]), 128) for co in cores) for h in range(2)]
    G = H[0] + H[1]
    meta = dict(NV=NV, G=G, ks=[], has_x=[], wait_wbs=[])
    if G == 0:
        for co in cores:
            co.update(idx=co["midx"], W=None, cnts=None)
        return uniq, cores, meta
    group_defs = []   # (half, start_within_half)
    for h in range(2):
        for k in range(H[h]):
            group_defs.append((h, k * 128))
            meta["wait_wbs"].append(WB_OF_HALF[h])
    for co in cores:
        co["rows_g"] = [co["halves"][h][st:st + 128] for h, st in group_defs]
        co["pairs"] = []
        co["xpairs"] = []
        for g in range(G):
            prs, xprs = [], []
            for r_i, row in enumerate(co["rows_g"][g]):
                row = int(row)
                for d in range(1, N_ITER + 1):
                    cv = co["Cc"][row, d]
                    if cv == 0:
                        continue
                    loc = row - d
                    if loc >= 0:
                        prs.append((r_i, _row_to_pk(loc), cv))
                    else:
                        xprs.append((r_i, co["s0"] + loc, cv))
            co["pairs"].append(prs)
            co["xpairs"].append(xprs)

    # SPMD program structure = union over cores
    ks = []
    has_x = []
    for g in range(G):
        used = sorted({pk[1] for co in cores for (_, pk, _) in co["pairs"][g]})
        ks.append(used)
        has_x.append(any(co["xpairs"][g] for co in cores))
        assert all(len(co["xpairs"][g]) <= 128 for co in cores)
    meta["ks"], meta["has_x"] = ks, has_x
    wbases = []
    wc = 0
    for g in range(G):
        wbases.append(wc)
        wc += (len(ks[g]) + (1 if has_x[g] else 0)) * 128
    meta["wbases"], meta["wcols"] = wbases, wc
    gxs = np.cumsum([0] + [1 if h else 0 for h in has_x])
    meta["gx"] = [int(gxs[g]) if has_x[g] else None for g in range(G)]
    meta["n_x"] = int(gxs[-1])

    for co in cores:
        b, s0 = co["b"], co["s0"]
        W = np.zeros((128, wc), dtype=BF16)
        xidx = np.zeros((128, meta["n_x"] * 8), dtype=np.int16)
        # one combined scatter for all groups: slot g*128+j adds delta[j,g,:]
        # to out[tgt].  Interior pad slots (j >= len(rg), non-final group)
        # target row 0 and carry an exactly-zero delta (their W columns are
        # all zero), so they are harmless; the final group pads trailing -1.
        tgts = np.full(G * 128, -1, dtype=np.int64)
        for g in range(G):
            base = wbases[g]
            kpos = {k: i for i, k in enumerate(ks[g])}
            for (r_i, (p, k), cv) in co["pairs"][g]:
                W[p, base + kpos[k] * 128 + r_i] = cv
            if has_x[g]:
                xbase = base + len(ks[g]) * 128
                xvals = np.zeros(128, dtype=np.int64)   # pads read row 0
                for xs, (r_i, gloc, cv) in enumerate(co["xpairs"][g]):
                    xvals[xs] = ridx[b, gloc]
                    W[xs, xbase + r_i] = cv
                xidx[:, meta["gx"][g] * 8:(meta["gx"][g] + 1) * 8] = \
                    _wrap16(xvals, 8)
            rg = co["rows_g"][g]
            if g < G - 1:
                tgts[g * 128:(g + 1) * 128] = 0
            tgts[g * 128:g * 128 + len(rg)] = rg
        sidx = _wrap16(tgts, G * 8)
        cnts = np.array([[(G - 1) * 128 + len(co["rows_g"][G - 1])]],
                        dtype=np.int32)
        co.update(idx=np.concatenate([co["midx"], sidx, xidx], axis=1),
                  W=W, cnts=cnts)
    return uniq, cores, meta


def _build_program(NV, G, ks, has_x, wait_wbs, wbases, wcols, gx, n_x):
    import concourse.bacc as bacc
    import concourse.mybir as mybir
    from concourse.library_config import mlp

    f32, i16, bf16 = mybir.dt.float32, mybir.dt.int16, mybir.dt.bfloat16

    nc = bacc.Bacc("TRN2", target_bir_lowering=False, debug=False,
                   enable_asserts=False, num_devices=N_CORES)
    SOFS = CPC_OFF[-1]
    XOFS = SOFS + G * 8
    icols = XOFS + n_x * 8
    table = nc.dram_tensor("table", [NV, D], bf16, kind="ExternalInput")
    idx_d = nc.dram_tensor("idx", [128, icols], i16, kind="ExternalInput")
    out_d = nc.dram_tensor("out", [RPC, D], bf16, kind="ExternalOutput")
    if G:
        w_d = nc.dram_tensor("W", [128, wcols], bf16, kind="ExternalInput")
        cnts_d = nc.dram_tensor("cnts", [1, 1], mybir.dt.int32,
                                kind="ExternalInput")

    from contextlib import ExitStack
    with ExitStack() as st:
        mbuf = st.enter_context(nc.sbuf_tensor("mbuf", [128, N_SLOTS, D], bf16))
        idx_s = st.enter_context(nc.sbuf_tensor("idx_s", [128, icols], i16))
        idx_sem = st.enter_context(nc.semaphore("idx_sem"))
        aux_sem = st.enter_context(nc.semaphore("aux_sem"))
        g_sems = [st.enter_context(nc.semaphore(f"g_sem{c}")) for c in range(N_CHUNKS)]
        w_sems = [st.enter_context(nc.semaphore(f"w_sem{c}")) for c in range(N_CHUNKS)]
        if G:
            w_s = st.enter_context(nc.sbuf_tensor("w_s", [128, wcols], bf16))
            cnts_s = st.enter_context(
                nc.sbuf_tensor("cnts_s", [1, 1], mybir.dt.int32))
            delta = st.enter_context(nc.sbuf_tensor("delta", [128, G, D], bf16))
            if n_x:
                xbuf = st.enter_context(nc.sbuf_tensor("xbuf", [128, n_x, D], bf16))
                x_sems = [st.enter_context(nc.semaphore(f"x_sem{i}"))
                          for i in range(n_x)]
            # one PSUM accumulator (4 banks) per in-flight group; groups >2
            # reuse banks after the delta copy drains them (d_sem ordering)
            ps = [st.enter_context(nc.psum_tensor(f"ps{i}", [128, D], f32))
                  for i in range(min(G, 2))]
            nreg = st.enter_context(nc.gpsimd.register("nreg"))
            mm_sem = st.enter_context(nc.semaphore("mm_sem"))
            d_sem = st.enter_context(nc.semaphore("d_sem"))
            s_sem = st.enter_context(nc.semaphore("s_sem"))
        block = st.enter_context(nc.Block())

        # per-chunk HWDGE writebacks, alternating the two rings: each ring
        # cycle costs ~10us of issue+completion latency beyond the transfer,
        # but per-chunk deps let the early writebacks overlap the remaining
        # gather drains (the SDMA engines round-robin fairly across queues)
        def writeback(eng, ci):
            wg, gb, ns = CHUNKS[ci]
            st_r, en_r, gpw, s0 = WBG[wg]
            eng.wait_ge(g_sems[ci], 16)
            dst = out_d[st_r:en_r, :].rearrange(
                "(p g) e -> p g e", g=gpw)[:, gb:gb + ns, :]
            eng.dma_start(dst, mbuf[:, s0 + gb:s0 + gb + ns, :]).then_inc(
                w_sems[ci], 16)

        @block.sync
        def _(sync):
            sync.dma_start(idx_s[:, :], idx_d[:, :]).then_inc(idx_sem, 16)
            if G:
                sync.dma_start(w_s[:, :], w_d[:, :]).then_inc(aux_sem, 16)
                sync.dma_start(cnts_s[:, :], cnts_d[:, :]).then_inc(aux_sem, 16)
            for ci in range(0, N_CHUNKS, 2):
                writeback(sync, ci)

        @block.scalar
        def _(scalar):
            for ci in range(1, N_CHUNKS, 2):
                writeback(scalar, ci)

        @block.gpsimd
        def _(gp):
            gp.load_library(mlp)
            gp.wait_ge(idx_sem, 16)

            def main_gather(ci):
                wg, gb, ns = CHUNKS[ci]
                s0 = WBG[wg][3] + gb
                cs = CHUNK_SIZES[ci]
                gp.dma_gather(mbuf[:, s0:s0 + ns, :], table[:, :],
                              idx_s[:, CPC_OFF[ci]:CPC_OFF[ci + 1]],
                              cs, cs, D,
                              single_packet=False).then_inc(g_sems[ci], 16)

            main_gather(0)
            main_gather(1)
            main_gather(2)
            for i in range(n_x):
                gp.dma_gather(xbuf[:, i:i + 1, :], table[:, :],
                              idx_s[:, XOFS + i * 8:XOFS + (i + 1) * 8],
                              128, 128, D,
                              single_packet=False).then_inc(x_sems[i], 16)
            main_gather(3)
            if G:
                # one combined scatter_add for all groups: src delta[:, :, :]
                # matches slot g*128+j <-> delta[j, g, :]
                gp.wait_ge(aux_sem, 32)                 # cnts loaded
                for ci in range(N_CHUNKS):              # target rows written
                    gp.wait_ge(w_sems[ci], 16)
                gp.wait_ge(d_sem, G)                    # deltas ready
                gp.reg_load(nreg, cnts_s[0:1, 0:1])
                gp.dma_scatter_add(out_d[:, :], delta[:, :, :],
                                   idx_s[:, SOFS:SOFS + G * 8],
                                   G * 128, nreg, D,
                                   single_packet=False).then_inc(s_sem, 16)
                gp.wait_ge(s_sem, 16)

        @block.tensor
        def _(te):
            if not G:
                return
            te.wait_ge(aux_sem, 32)                     # W loaded
            waited = set()

            def mwait(sem, v, key):
                if key not in waited:
                    te.wait_ge(sem, v)
                    waited.add(key)

            for g in range(G):
                if g >= 2:                              # PSUM banks reused
                    te.wait_ge(d_sem, g - 1)
                base, pg = wbases[g], ps[g % 2]
                n_mm = len(ks[g]) + (1 if has_x[g] else 0)
                for ki, k in enumerate(ks[g]):
                    mwait(g_sems[SLOT_CHUNK[k]], 16, ("g", SLOT_CHUNK[k]))
                    for f in range(0, D, FSTEP):
                        ins = te.matmul(pg[:, f:f + FSTEP],
                                        w_s[:, base + ki * 128:base + (ki + 1) * 128],
                                        mbuf[:, k, f:f + FSTEP],
                                        start=(ki == 0), stop=(ki == n_mm - 1))
                        if ki == n_mm - 1:
                            ins.then_inc(mm_sem, 1)
                if has_x[g]:
                    xi = gx[g]
                    mwait(x_sems[xi], 16, ("x", xi))
                    xbase = base + len(ks[g]) * 128
                    for f in range(0, D, FSTEP):
                        te.matmul(pg[:, f:f + FSTEP],
                                  w_s[:, xbase:xbase + 128],
                                  xbuf[:, xi, f:f + FSTEP],
                                  start=(len(ks[g]) == 0),
                                  stop=True).then_inc(mm_sem, 1)

        @block.vector
        def _(v):
            if not G:
                return
            # drain each group's PSUM accumulator to a bf16 delta tile
            for g in range(G):
                v.wait_ge(mm_sem, (D // FSTEP) * (g + 1))
                v.tensor_scalar_mul(delta[:, g, :], ps[g % 2][:, :], 1.0
                                    ).then_inc(d_sem, 1)

    nc.compile()
    return nc


_CACHE = {}
_LAST_RESULT = None


def kernel(x, emb_table):
    global _LAST_RESULT
    from concourse.bass_utils import run_bass_kernel_spmd

    x_np = np.asarray(x)
    emb_np = np.asarray(emb_table, dtype=np.float32)
    uniq, cores, meta = _prepare(x_np)
    table_bf = np.ascontiguousarray(emb_np[uniq].astype(BF16))

    key = (meta["NV"], meta["G"], tuple(tuple(k) for k in meta["ks"]),
           tuple(meta["has_x"]),
           tuple(tuple(w) for w in meta["wait_wbs"]))
    if key not in _CACHE:
        _CACHE[key] = _build_program(
            meta["NV"], meta["G"], meta["ks"], meta["has_x"],
            meta["wait_wbs"], meta.get("wbases"), meta.get("wcols"),
            meta.get("gx"), meta.get("n_x", 0))
    nc = _CACHE[key]

    in_maps = []
    for co in cores:
        m = {"table": table_bf, "idx": co["idx"]}
        if meta["G"]:
            m["W"] = co["W"]
            m["cnts"] = co["cnts"]
        in_maps.append(m)

    res = run_bass_kernel_spmd(nc, in_maps, core_ids=list(range(N_CORES)))
    _LAST_RESULT = res
    full = np.empty((B, S, D), dtype=np.float32)
    for c in range(N_CORES):
        b, h = c // 2, c % 2
        full[b, h * RPC:(h + 1) * RPC, :] = res.results[c]["out"].astype(np.float32)
    return full


# revision 30
# speedup vs baseline: 1.0192x; 1.0192x over previous
"""Trainium2 Bass kernel for nn_BlankEmbedding (embedding gather + blank-run scan).

Math: the reference computes e = emb_table[x], then runs 8 iterations of
    pos = shift_right(pos); acc = shift_right(acc); out = out + acc; acc = out*pos
starting from pos = is_preblank.  Unrolling, out[i] = sum_{d=0..8} C[i,d]*e[i-d]
with banded integer coefficients C depending only on x; C[i,0] == 1 and rows
with any C[i,d>0] != 0 are rare (~1/128 blank density -> ~136 rows per core).

Device strategy (per core, 2048 of the 16384 rows, data-parallel over B*S):
  * the deduplicated table is converted to bf16 on the host; the device works
    in bf16 end-to-end (gather, writeback, matmul, scatter-add) and the host
    upcasts the result to fp32.  bf16 quantization is ~2^-9 relative error,
    far under the 2e-2 gate, and halves every DMA byte.
  * main path: dma_gather the core's 2048 rows (5 SWDGE chunks into a fully
    resident SBUF buffer - 64KB/partition in bf16, so gathers never wait on
    writebacks) and write each chunk out with a strided HWDGE DMA,
    alternating the sync/scalar rings.
  * affected rows (grouped <=128 per output half): their band rows e[i-d] are
    already in the main SBUF buffer, so the correction
        delta[r, :] = sum_d C[r,d] * e[r-d]
    is computed on the (otherwise idle) tensor engine as
        delta = sum_k W_k[p, r] . mbuf[p, k, :]
    with a host-built sparse bf16 coefficient matrix W (one [128,128] chunk
    per mbuf free slot k), accumulated in fp32 PSUM.  Band rows that fall
    before the core's row range (cross-half) come from one small padded
    dma_gather (xbuf) with its own W chunk.  The PSUM delta is copied to
    bf16 by DVE and dma_scatter_add-ed onto the output rows as soon as the
    covering chunk writebacks land.  No per-depth gathers, no DVE chains.

Host side computes index lists / coefficient matrices from x ([B,S] int
ops), the bf16 table conversion, and reassembles + upcasts the outputs.
"""

import numpy as np
import ml_dtypes

BF16 = ml_dtypes.bfloat16

B, S, D = 4, 4096, 2048
N_CORES = 8
RPC = (B * S) // N_CORES          # rows per core = 2048
# uneven chunks: the small final chunk makes the last writeback (which gates
# the final scatter_add) complete quickly after the gather stream drains
CHUNK_SIZES = [512, 512, 512, 384, 128]
N_CHUNKS = len(CHUNK_SIZES)
CHUNK_OFF = [sum(CHUNK_SIZES[:i]) for i in range(N_CHUNKS + 1)]
GPPS = [cs // 128 for cs in CHUNK_SIZES]  # rows per partition per chunk
MOFF = [o // 128 for o in CHUNK_OFF]      # chunk offset in mbuf free slots
CPCS = [cs // 16 for cs in CHUNK_SIZES]   # idx columns per chunk
CPC_OFF = [sum(CPCS[:i]) for i in range(N_CHUNKS + 1)]
NBLANK_IDS = 16
N_ITER = 8
BAND = N_ITER + 1                 # out[i] depends on e[i-8..i]
FSTEP = 512                       # one PSUM bank of fp32 per matmul


def _cdiv(a, b):
    return (a + b - 1) // b


def _chunk_of_slot(k):
    for ch in range(N_CHUNKS):
        if MOFF[ch] <= k < MOFF[ch + 1]:
            return ch
    raise ValueError(k)


def _row_to_pk(l):
    """mbuf location (partition, free slot) of local row l (matches midx)."""
    for ch in range(N_CHUNKS):
        if CHUNK_OFF[ch] <= l < CHUNK_OFF[ch + 1]:
            w = l - CHUNK_OFF[ch]
            return w // GPPS[ch], MOFF[ch] + w % GPPS[ch]
    raise ValueError(l)


def _compute_coeffs(x):
    """C[b, s, d] for d=0..8 (float64 holds small ints exactly), per batch row."""
    b, s = x.shape
    blank = ((x >= 0) & (x < NBLANK_IDS)).astype(np.float64)
    shift_r = lambda t: np.concatenate([np.zeros_like(t[:, :1]), t[:, :-1]], axis=1)
    first = np.maximum(blank - shift_r(blank), 0.0)
    m = np.concatenate([first[:, 1:], np.zeros_like(first[:, :1])], axis=1)  # preblank
    C = np.zeros((b, s, BAND))
    C[:, :, 0] = 1.0
    for k in range(1, N_ITER + 1):
        m_k = np.zeros_like(m)
        m_k[:, k:] = m[:, :-k]                       # m[i-k]
        Cs = np.zeros_like(C)
        Cs[:, 1:, 1:] = C[:, :-1, :-1]               # C[i-1, d-1]
        C = C + m_k[:, :, None] * Cs
    return C


def _wrap16(vals, ncols):
    """Wrap a 1-D index list into the [128, ncols] int16 layout the SWDGE
    gather/scatter ucode expects: slot j at [j % 16, j // 16], and the 16-row
    block replicated across all eight 16-partition Q7 core groups."""
    blk = np.zeros((16, ncols), dtype=np.int16)
    v = np.asarray(vals, dtype=np.int16)
    blk[np.arange(len(v)) % 16, np.arange(len(v)) // 16] = v
    return np.tile(blk, (8, 1))


def _prepare(x_np):
    """All host-side index/coefficient prep. Returns per-core arrays + meta."""
    uniq, inv = np.unique(x_np, return_inverse=True)
    ridx = inv.reshape(x_np.shape).astype(np.int64)   # x remapped to table rows
    NV = len(uniq)
    assert NV <= 32767, "int16 gather index overflow"

    C = _compute_coeffs(x_np)
    assert (np.abs(C) <= 256).all(), "coefficients exceed bf16-exact range"
    aff = (C[:, :, 1:] != 0).any(axis=2)              # [B,S]

    cores = []
    for c in range(N_CORES):
        b, h = c // 2, c % 2
        s0 = h * RPC
        # main gather indices, permuted so SBUF partition p holds rows p*gpp+g
        midx = np.zeros((128, CPC_OFF[-1]), dtype=np.int16)
        for ch in range(N_CHUNKS):
            cs, gpp = CHUNK_SIZES[ch], GPPS[ch]
            j = np.arange(cs)
            slots = ridx[b, s0 + (j % 128) * gpp + (j // 128) + CHUNK_OFF[ch]]
            midx[:, CPC_OFF[ch]:CPC_OFF[ch + 1]] = _wrap16(slots, CPCS[ch])

        # affected rows split by output half: the group over rows < RPC/2 can
        # scatter as soon as the first two chunk writebacks land
        rows_all = np.nonzero(aff[b, s0:s0 + RPC])[0]
        Cc = C[b, s0:s0 + RPC]                        # [RPC, 9] (local view)
        halves = [rows_all[(rows_all >= hh * (RPC // 2))
                           & (rows_all < (hh + 1) * (RPC // 2))]
                  for hh in range(2)]
        cores.append(dict(b=b, s0=s0, halves=halves, Cc=Cc, midx=midx))

    # groups per half = max over cores; group g of half h waits for the
    # writebacks covering that half
    H = [max(_cdiv(len(co["halves"][h]), 128) for co in cores) for h in range(2)]
    G = H[0] + H[1]
    meta = dict(NV=NV, G=G, ks=[], has_x=[], wait_chunks=[])
    if G == 0:
        for co in cores:
            co.update(idx=co["midx"], W=None, cnts=None)
        return uniq, cores, meta
    group_defs = []   # (half, start_within_half)
    for h in range(2):
        for k in range(H[h]):
            group_defs.append((h, k * 128))
            meta["wait_chunks"].append(2 if h == 0 else N_CHUNKS)
    for co in cores:
        co["rows_g"] = [co["halves"][h][st:st + 128] for h, st in group_defs]
        # pair lists per group: (target slot r, mbuf (p,k) or xbuf slot, coef)
        co["pairs"] = []
        co["xpairs"] = []
        for g in range(G):
            prs, xprs = [], []
            for r_i, row in enumerate(co["rows_g"][g]):
                row = int(row)
                for d in range(1, N_ITER + 1):
                    cv = co["Cc"][row, d]
                    if cv == 0:
                        continue
                    loc = row - d
                    if loc >= 0:
                        prs.append((r_i, _row_to_pk(loc), cv))
                    else:
                        xprs.append((r_i, co["s0"] + loc, cv))
            co["pairs"].append(prs)
            co["xpairs"].append(xprs)

    # SPMD program structure = union over cores
    ks = []
    has_x = []
    for g in range(G):
        used = sorted({pk[1] for co in cores for (_, pk, _) in co["pairs"][g]})
        ks.append(used)
        has_x.append(any(co["xpairs"][g] for co in cores))
        assert all(len(co["xpairs"][g]) <= 128 for co in cores)
    meta["ks"], meta["has_x"] = ks, has_x
    wbases = []
    wc = 0
    for g in range(G):
        wbases.append(wc)
        wc += (len(ks[g]) + (1 if has_x[g] else 0)) * 128
    meta["wbases"], meta["wcols"] = wbases, wc
    gxs = np.cumsum([0] + [1 if h else 0 for h in has_x])
    meta["gx"] = [int(gxs[g]) if has_x[g] else None for g in range(G)]
    meta["n_x"] = int(gxs[-1])

    for co in cores:
        b, s0 = co["b"], co["s0"]
        W = np.zeros((128, wc), dtype=BF16)
        sidx = np.zeros((128, G * 8), dtype=np.int16)
        xidx = np.zeros((128, meta["n_x"] * 8), dtype=np.int16)
        cnts = np.zeros((1, G), dtype=np.int32)
        for g in range(G):
            base = wbases[g]
            kpos = {k: i for i, k in enumerate(ks[g])}
            for (r_i, (p, k), cv) in co["pairs"][g]:
                W[p, base + kpos[k] * 128 + r_i] = cv
            if has_x[g]:
                xbase = base + len(ks[g]) * 128
                xvals = np.zeros(128, dtype=np.int64)   # pads read row 0
                for xs, (r_i, gloc, cv) in enumerate(co["xpairs"][g]):
                    xvals[xs] = ridx[b, gloc]
                    W[xs, xbase + r_i] = cv
                xidx[:, meta["gx"][g] * 8:(meta["gx"][g] + 1) * 8] = \
                    _wrap16(xvals, 8)
            rg = co["rows_g"][g]
            tgts = np.full(128, -1, dtype=np.int64)   # trailing -1 are skipped
            tgts[:len(rg)] = rg
            sidx[:, g * 8:(g + 1) * 8] = _wrap16(tgts, 8)
            cnts[0, g] = len(rg)
        co.update(idx=np.concatenate([co["midx"], sidx, xidx], axis=1),
                  W=W, cnts=cnts)
    return uniq, cores, meta


def _build_program(NV, G, ks, has_x, wait_chunks, wbases, wcols, gx, n_x):
    import concourse.bacc as bacc
    import concourse.mybir as mybir
    from concourse.library_config import mlp

    f32, i16, bf16 = mybir.dt.float32, mybir.dt.int16, mybir.dt.bfloat16

    nc = bacc.Bacc("TRN2", target_bir_lowering=False, debug=False,
                   enable_asserts=False, num_devices=N_CORES)
    SOFS = CPC_OFF[-1]
    XOFS = SOFS + G * 8
    icols = XOFS + n_x * 8
    table = nc.dram_tensor("table", [NV, D], bf16, kind="ExternalInput")
    idx_d = nc.dram_tensor("idx", [128, icols], i16, kind="ExternalInput")
    out_d = nc.dram_tensor("out", [RPC, D], bf16, kind="ExternalOutput")
    if G:
        w_d = nc.dram_tensor("W", [128, wcols], bf16, kind="ExternalInput")
        cnts_d = nc.dram_tensor("cnts", [1, G], mybir.dt.int32,
                                kind="ExternalInput")

    from contextlib import ExitStack
    with ExitStack() as st:
        # all 2048 rows stay resident (bf16: 64KB/partition), so gathers never
        # wait on writebacks
        mbuf = st.enter_context(nc.sbuf_tensor("mbuf", [128, MOFF[-1], D], bf16))
        idx_s = st.enter_context(nc.sbuf_tensor("idx_s", [128, icols], i16))
        idx_sem = st.enter_context(nc.semaphore("idx_sem"))
        g_sems = [st.enter_context(nc.semaphore(f"g_sem{c}")) for c in range(N_CHUNKS)]
        w_sems = [st.enter_context(nc.semaphore(f"w_sem{c}")) for c in range(N_CHUNKS)]
        if G:
            w_s = st.enter_context(nc.sbuf_tensor("w_s", [128, wcols], bf16))
            cnts_s = st.enter_context(
                nc.sbuf_tensor("cnts_s", [1, G], mybir.dt.int32))
            delta = st.enter_context(nc.sbuf_tensor("delta", [128, G, D], bf16))
            if n_x:
                xbuf = st.enter_context(nc.sbuf_tensor("xbuf", [128, n_x, D], bf16))
                x_sems = [st.enter_context(nc.semaphore(f"x_sem{i}"))
                          for i in range(n_x)]
            # one PSUM accumulator (4 banks) per in-flight group; groups >2
            # reuse banks after the delta copy drains them (d_sem ordering)
            ps = [st.enter_context(nc.psum_tensor(f"ps{i}", [128, D], f32))
                  for i in range(min(G, 2))]
            nreg = st.enter_context(nc.gpsimd.register("nreg"))
            mm_sem = st.enter_context(nc.semaphore("mm_sem"))
            d_sem = st.enter_context(nc.semaphore("d_sem"))
            s_sem = st.enter_context(nc.semaphore("s_sem"))
        block = st.enter_context(nc.Block())
        n_idx_dmas = 3 if G else 1

        def writeback(eng, ch):
            eng.wait_ge(g_sems[ch], 16)
            dst = out_d[CHUNK_OFF[ch]:CHUNK_OFF[ch + 1], :].rearrange(
                "(p g) e -> p g e", g=GPPS[ch])
            eng.dma_start(dst, mbuf[:, MOFF[ch]:MOFF[ch + 1], :]).then_inc(
                w_sems[ch], 16)

        @block.sync
        def _(sync):
            sync.dma_start(idx_s[:, :], idx_d[:, :]).then_inc(idx_sem, 16)
            if G:
                sync.dma_start(w_s[:, :], w_d[:, :]).then_inc(idx_sem, 16)
                sync.dma_start(cnts_s[:, :], cnts_d[:, :]).then_inc(idx_sem, 16)
            for ch in range(0, N_CHUNKS, 2):
                writeback(sync, ch)

        @block.scalar
        def _(scalar):
            for ch in range(1, N_CHUNKS, 2):
                writeback(scalar, ch)

        @block.gpsimd
        def _(gp):
            gp.load_library(mlp)
            gp.wait_ge(idx_sem, 16 * n_idx_dmas)

            def main_gather(ch):
                cs = CHUNK_SIZES[ch]
                gp.dma_gather(mbuf[:, MOFF[ch]:MOFF[ch + 1], :], table[:, :],
                              idx_s[:, CPC_OFF[ch]:CPC_OFF[ch + 1]],
                              cs, cs, D,
                              single_packet=False).then_inc(g_sems[ch], 16)

            main_gather(0)
            main_gather(1)
            # the small cross-half gathers go early so the matmul chains that
            # need them can close mid-stream
            for i in range(n_x):
                gp.dma_gather(xbuf[:, i:i + 1, :], table[:, :],
                              idx_s[:, XOFS + i * 8:XOFS + (i + 1) * 8],
                              128, 128, D,
                              single_packet=False).then_inc(x_sems[i], 16)
            for ch in range(2, N_CHUNKS):
                main_gather(ch)
            for g in range(G):
                for c in range(wait_chunks[g]):         # target rows written
                    gp.wait_ge(w_sems[c], 16)
                gp.wait_ge(d_sem, g + 1)                # delta ready
                gp.reg_load(nreg, cnts_s[0:1, g:g + 1])
                gp.dma_scatter_add(out_d[:, :], delta[:, g:g + 1, :],
                                   idx_s[:, SOFS + g * 8:SOFS + (g + 1) * 8],
                                   128, nreg, D,
                                   single_packet=False).then_inc(s_sem, 16)
            if G:
                gp.wait_ge(s_sem, 16 * G)

        @block.tensor
        def _(te):
            if not G:
                return
            te.wait_ge(idx_sem, 16 * n_idx_dmas)
            waited = set()

            def mwait(sem, v, key):
                if key not in waited:
                    te.wait_ge(sem, v)
                    waited.add(key)

            for g in range(G):
                if g >= 2:                              # PSUM banks reused
                    te.wait_ge(d_sem, g - 1)
                base, pg = wbases[g], ps[g % 2]
                n_mm = len(ks[g]) + (1 if has_x[g] else 0)
                for ki, k in enumerate(ks[g]):
                    mwait(g_sems[_chunk_of_slot(k)], 16, ("g", _chunk_of_slot(k)))
                    for f in range(0, D, FSTEP):
                        ins = te.matmul(pg[:, f:f + FSTEP],
                                        w_s[:, base + ki * 128:base + (ki + 1) * 128],
                                        mbuf[:, k, f:f + FSTEP],
                                        start=(ki == 0), stop=(ki == n_mm - 1))
                        if ki == n_mm - 1:
                            ins.then_inc(mm_sem, 1)
                if has_x[g]:
                    xi = gx[g]
                    mwait(x_sems[xi], 16, ("x", xi))
                    xbase = base + len(ks[g]) * 128
                    for f in range(0, D, FSTEP):
                        te.matmul(pg[:, f:f + FSTEP],
                                  w_s[:, xbase:xbase + 128],
                                  xbuf[:, xi, f:f + FSTEP],
                                  start=(len(ks[g]) == 0),
                                  stop=True).then_inc(mm_sem, 1)

        @block.vector
        def _(v):
            if not G:
                return
            # drain each group's PSUM accumulator to a bf16 delta tile
            for g in range(G):
                v.wait_ge(mm_sem, (D // FSTEP) * (g + 1))
                v.tensor_scalar_mul(delta[:, g, :], ps[g % 2][:, :], 1.0
                                    ).then_inc(d_sem, 1)

    nc.compile()
    return nc


_CACHE = {}
_LAST_RESULT = None


def kernel(x, emb_table):
    global _LAST_RESULT
    from concourse.bass_utils import run_bass_kernel_spmd

    x_np = np.asarray(x)
    emb_np = np.asarray(emb_table, dtype=np.float32)
    uniq, cores, meta = _prepare(x_np)
    table_bf = np.ascontiguousarray(emb_np[uniq].astype(BF16))

    key = (meta["NV"], meta["G"], tuple(tuple(k) for k in meta["ks"]),
           tuple(meta["has_x"]), tuple(meta["wait_chunks"]))
    if key not in _CACHE:
        _CACHE[key] = _build_program(
            meta["NV"], meta["G"], meta["ks"], meta["has_x"],
            meta["wait_chunks"], meta.get("wbases"), meta.get("wcols"),
            meta.get("gx"), meta.get("n_x", 0))
    nc = _CACHE[key]

    in_maps = []
    for co in cores:
        m = {"table": table_bf, "idx": co["idx"]}
        if meta["G"]:
            m["W"] = co["W"]
            m["cnts"] = co["cnts"]
        in_maps.append(m)

    res = run_bass_kernel_spmd(nc, in_maps, core_ids=list(range(N_CORES)))
    _LAST_RESULT = res
    full = np.empty((B, S, D), dtype=np.float32)
    for c in range(N_CORES):
        b, h = c // 2, c % 2
        full[b, h * RPC:(h + 1) * RPC, :] = res.results[c]["out"].astype(np.float32)
    return full


# revision 31
# speedup vs baseline: 1.0590x; 1.0391x over previous
"""Trainium2 Bass kernel for nn_BlankEmbedding (embedding gather + blank-run scan).

Math: the reference computes e = emb_table[x], then runs 8 iterations of
    pos = shift_right(pos); acc = shift_right(acc); out = out + acc; acc = out*pos
starting from pos = is_preblank.  Unrolling, out[i] = sum_{d=0..8} C[i,d]*e[i-d]
with banded integer coefficients C depending only on x; C[i,0] == 1 and rows
with any C[i,d>0] != 0 are rare (~1/128 blank density -> ~136 rows per core).

Device strategy (per core, 2048 of the 16384 rows, data-parallel over B*S):
  * the deduplicated table is converted to bf16 on the host; the device works
    in bf16 end-to-end (gather, writeback, matmul, scatter-add) and the host
    upcasts the result to fp32.  bf16 quantization is ~2^-9 relative error,
    far under the 2e-2 gate, and halves every DMA byte.
  * main path: dma_gather the core's 2048 rows (5 SWDGE chunks into a fully
    resident SBUF buffer - 64KB/partition in bf16, so gathers never wait on
    writebacks) and write each chunk out with a strided HWDGE DMA,
    alternating the sync/scalar rings.
  * affected rows (grouped <=128 per output half): their band rows e[i-d] are
    already in the main SBUF buffer, so the correction
        delta[r, :] = sum_d C[r,d] * e[r-d]
    is computed on the (otherwise idle) tensor engine as
        delta = sum_k W_k[p, r] . mbuf[p, k, :]
    with a host-built sparse bf16 coefficient matrix W (one [128,128] chunk
    per mbuf free slot k), accumulated in fp32 PSUM.  Band rows that fall
    before the core's row range (cross-half) come from one small padded
    dma_gather (xbuf) with its own W chunk.  The PSUM delta is copied to
    bf16 by DVE and dma_scatter_add-ed onto the output rows as soon as the
    covering chunk writebacks land.  No per-depth gathers, no DVE chains.

Host side computes index lists / coefficient matrices from x ([B,S] int
ops), the bf16 table conversion, and reassembles + upcasts the outputs.
"""

import numpy as np
import ml_dtypes

BF16 = ml_dtypes.bfloat16

B, S, D = 4, 4096, 2048
N_CORES = 8
RPC = (B * S) // N_CORES          # rows per core = 2048
# uneven chunks: the small final chunk makes the last writeback (which gates
# the final scatter_add) complete quickly after the gather stream drains
CHUNK_SIZES = [512, 512, 512, 384, 128]
N_CHUNKS = len(CHUNK_SIZES)
CHUNK_OFF = [sum(CHUNK_SIZES[:i]) for i in range(N_CHUNKS + 1)]
GPPS = [cs // 128 for cs in CHUNK_SIZES]  # rows per partition per chunk
MOFF = [o // 128 for o in CHUNK_OFF]      # chunk offset in mbuf free slots
CPCS = [cs // 16 for cs in CHUNK_SIZES]   # idx columns per chunk
CPC_OFF = [sum(CPCS[:i]) for i in range(N_CHUNKS + 1)]
NBLANK_IDS = 16
N_ITER = 8
BAND = N_ITER + 1                 # out[i] depends on e[i-8..i]
FSTEP = 512                       # one PSUM bank of fp32 per matmul


def _cdiv(a, b):
    return (a + b - 1) // b


def _chunk_of_slot(k):
    for ch in range(N_CHUNKS):
        if MOFF[ch] <= k < MOFF[ch + 1]:
            return ch
    raise ValueError(k)


def _row_to_pk(l):
    """mbuf location (partition, free slot) of local row l (matches midx)."""
    for ch in range(N_CHUNKS):
        if CHUNK_OFF[ch] <= l < CHUNK_OFF[ch + 1]:
            w = l - CHUNK_OFF[ch]
            return w // GPPS[ch], MOFF[ch] + w % GPPS[ch]
    raise ValueError(l)


def _compute_coeffs(x):
    """C[b, s, d] for d=0..8 (float64 holds small ints exactly), per batch row."""
    b, s = x.shape
    blank = ((x >= 0) & (x < NBLANK_IDS)).astype(np.float64)
    shift_r = lambda t: np.concatenate([np.zeros_like(t[:, :1]), t[:, :-1]], axis=1)
    first = np.maximum(blank - shift_r(blank), 0.0)
    m = np.concatenate([first[:, 1:], np.zeros_like(first[:, :1])], axis=1)  # preblank
    C = np.zeros((b, s, BAND))
    C[:, :, 0] = 1.0
    for k in range(1, N_ITER + 1):
        m_k = np.zeros_like(m)
        m_k[:, k:] = m[:, :-k]                       # m[i-k]
        Cs = np.zeros_like(C)
        Cs[:, 1:, 1:] = C[:, :-1, :-1]               # C[i-1, d-1]
        C = C + m_k[:, :, None] * Cs
    return C


def _wrap16(vals, ncols):
    """Wrap a 1-D index list into the [128, ncols] int16 layout the SWDGE
    gather/scatter ucode expects: slot j at [j % 16, j // 16], and the 16-row
    block replicated across all eight 16-partition Q7 core groups."""
    blk = np.zeros((16, ncols), dtype=np.int16)
    v = np.asarray(vals, dtype=np.int16)
    blk[np.arange(len(v)) % 16, np.arange(len(v)) // 16] = v
    return np.tile(blk, (8, 1))


def _prepare(x_np):
    """All host-side index/coefficient prep. Returns per-core arrays + meta."""
    uniq, inv = np.unique(x_np, return_inverse=True)
    ridx = inv.reshape(x_np.shape).astype(np.int64)   # x remapped to table rows
    NV = len(uniq)
    assert NV <= 32767, "int16 gather index overflow"

    C = _compute_coeffs(x_np)
    assert (np.abs(C) <= 256).all(), "coefficients exceed bf16-exact range"
    aff = (C[:, :, 1:] != 0).any(axis=2)              # [B,S]

    cores = []
    for c in range(N_CORES):
        b, h = c // 2, c % 2
        s0 = h * RPC
        # main gather indices, permuted so SBUF partition p holds rows p*gpp+g
        midx = np.zeros((128, CPC_OFF[-1]), dtype=np.int16)
        for ch in range(N_CHUNKS):
            cs, gpp = CHUNK_SIZES[ch], GPPS[ch]
            j = np.arange(cs)
            slots = ridx[b, s0 + (j % 128) * gpp + (j // 128) + CHUNK_OFF[ch]]
            midx[:, CPC_OFF[ch]:CPC_OFF[ch + 1]] = _wrap16(slots, CPCS[ch])

        # affected rows split by output half: the group over rows < RPC/2 can
        # scatter as soon as the first two chunk writebacks land
        rows_all = np.nonzero(aff[b, s0:s0 + RPC])[0]
        Cc = C[b, s0:s0 + RPC]                        # [RPC, 9] (local view)
        halves = [rows_all[(rows_all >= hh * (RPC // 2))
                           & (rows_all < (hh + 1) * (RPC // 2))]
                  for hh in range(2)]
        cores.append(dict(b=b, s0=s0, halves=halves, Cc=Cc, midx=midx))

    # groups per half = max over cores; group g of half h waits for the
    # writebacks covering that half
    H = [max(_cdiv(len(co["halves"][h]), 128) for co in cores) for h in range(2)]
    G = H[0] + H[1]
    meta = dict(NV=NV, G=G, ks=[], has_x=[], wait_chunks=[])
    if G == 0:
        for co in cores:
            co.update(idx=co["midx"], W=None, cnts=None)
        return uniq, cores, meta
    group_defs = []   # (half, start_within_half)
    for h in range(2):
        for k in range(H[h]):
            group_defs.append((h, k * 128))
            meta["wait_chunks"].append(2 if h == 0 else N_CHUNKS)
    for co in cores:
        co["rows_g"] = [co["halves"][h][st:st + 128] for h, st in group_defs]
        # pair lists per group: (target slot r, mbuf (p,k) or xbuf slot, coef)
        co["pairs"] = []
        co["xpairs"] = []
        for g in range(G):
            prs, xprs = [], []
            for r_i, row in enumerate(co["rows_g"][g]):
                row = int(row)
                for d in range(1, N_ITER + 1):
                    cv = co["Cc"][row, d]
                    if cv == 0:
                        continue
                    loc = row - d
                    if loc >= 0:
                        prs.append((r_i, _row_to_pk(loc), cv))
                    else:
                        xprs.append((r_i, co["s0"] + loc, cv))
            co["pairs"].append(prs)
            co["xpairs"].append(xprs)

    # SPMD program structure = union over cores
    ks = []
    has_x = []
    for g in range(G):
        used = sorted({pk[1] for co in cores for (_, pk, _) in co["pairs"][g]})
        ks.append(used)
        has_x.append(any(co["xpairs"][g] for co in cores))
        assert all(len(co["xpairs"][g]) <= 128 for co in cores)
    meta["ks"], meta["has_x"] = ks, has_x
    wbases = []
    wc = 0
    for g in range(G):
        wbases.append(wc)
        wc += (len(ks[g]) + (1 if has_x[g] else 0)) * 128
    meta["wbases"], meta["wcols"] = wbases, wc
    gxs = np.cumsum([0] + [1 if h else 0 for h in has_x])
    meta["gx"] = [int(gxs[g]) if has_x[g] else None for g in range(G)]
    meta["n_x"] = int(gxs[-1])

    for co in cores:
        b, s0 = co["b"], co["s0"]
        W = np.zeros((128, wc), dtype=BF16)
        sidx = np.zeros((128, G * 8), dtype=np.int16)
        xidx = np.zeros((128, meta["n_x"] * 8), dtype=np.int16)
        cnts = np.zeros((1, G), dtype=np.int32)
        for g in range(G):
            base = wbases[g]
            kpos = {k: i for i, k in enumerate(ks[g])}
            for (r_i, (p, k), cv) in co["pairs"][g]:
                W[p, base + kpos[k] * 128 + r_i] = cv
            if has_x[g]:
                xbase = base + len(ks[g]) * 128
                xvals = np.zeros(128, dtype=np.int64)   # pads read row 0
                for xs, (r_i, gloc, cv) in enumerate(co["xpairs"][g]):
                    xvals[xs] = ridx[b, gloc]
                    W[xs, xbase + r_i] = cv
                xidx[:, meta["gx"][g] * 8:(meta["gx"][g] + 1) * 8] = \
                    _wrap16(xvals, 8)
            rg = co["rows_g"][g]
            tgts = np.full(128, -1, dtype=np.int64)   # trailing -1 are skipped
            tgts[:len(rg)] = rg
            sidx[:, g * 8:(g + 1) * 8] = _wrap16(tgts, 8)
            cnts[0, g] = len(rg)
        co.update(idx=np.concatenate([co["midx"], sidx, xidx], axis=1),
                  W=W, cnts=cnts)
    return uniq, cores, meta


def _build_program(NV, G, ks, has_x, wait_chunks, wbases, wcols, gx, n_x):
    import concourse.bacc as bacc
    import concourse.mybir as mybir
    from concourse.library_config import mlp

    f32, i16, bf16 = mybir.dt.float32, mybir.dt.int16, mybir.dt.bfloat16

    nc = bacc.Bacc("TRN2", target_bir_lowering=False, debug=False,
                   enable_asserts=False, num_devices=N_CORES)
    SOFS = CPC_OFF[-1]
    XOFS = SOFS + G * 8
    icols = XOFS + n_x * 8
    table = nc.dram_tensor("table", [NV, D], bf16, kind="ExternalInput")
    idx_d = nc.dram_tensor("idx", [128, icols], i16, kind="ExternalInput")
    out_d = nc.dram_tensor("out", [RPC, D], bf16, kind="ExternalOutput")
    if G:
        w_d = nc.dram_tensor("W", [128, wcols], bf16, kind="ExternalInput")
        cnts_d = nc.dram_tensor("cnts", [1, G], mybir.dt.int32,
                                kind="ExternalInput")

    from contextlib import ExitStack
    with ExitStack() as st:
        # all 2048 rows stay resident (bf16: 64KB/partition), so gathers never
        # wait on writebacks
        mbuf = st.enter_context(nc.sbuf_tensor("mbuf", [128, MOFF[-1], D], bf16))
        idx_s = st.enter_context(nc.sbuf_tensor("idx_s", [128, icols], i16))
        idx_sem = st.enter_context(nc.semaphore("idx_sem"))
        g_sems = [st.enter_context(nc.semaphore(f"g_sem{c}")) for c in range(N_CHUNKS)]
        w_sems = [st.enter_context(nc.semaphore(f"w_sem{c}")) for c in range(N_CHUNKS)]
        if G:
            w_s = st.enter_context(nc.sbuf_tensor("w_s", [128, wcols], bf16))
            cnts_s = st.enter_context(
                nc.sbuf_tensor("cnts_s", [1, G], mybir.dt.int32))
            delta = st.enter_context(nc.sbuf_tensor("delta", [128, G, D], bf16))
            if n_x:
                xbuf = st.enter_context(nc.sbuf_tensor("xbuf", [128, n_x, D], bf16))
                x_sems = [st.enter_context(nc.semaphore(f"x_sem{i}"))
                          for i in range(n_x)]
            # one PSUM accumulator (4 banks) per in-flight group; groups >2
            # reuse banks after the delta copy drains them (d_sem ordering)
            ps = [st.enter_context(nc.psum_tensor(f"ps{i}", [128, D], f32))
                  for i in range(min(G, 2))]
            nreg = st.enter_context(nc.gpsimd.register("nreg"))
            mm_sem = st.enter_context(nc.semaphore("mm_sem"))
            d_sem = st.enter_context(nc.semaphore("d_sem"))
            s_sem = st.enter_context(nc.semaphore("s_sem"))
        block = st.enter_context(nc.Block())
        n_idx_dmas = 3 if G else 1

        def writeback(eng, ch):
            eng.wait_ge(g_sems[ch], 16)
            dst = out_d[CHUNK_OFF[ch]:CHUNK_OFF[ch + 1], :].rearrange(
                "(p g) e -> p g e", g=GPPS[ch])
            eng.dma_start(dst, mbuf[:, MOFF[ch]:MOFF[ch + 1], :]).then_inc(
                w_sems[ch], 16)

        @block.sync
        def _(sync):
            sync.dma_start(idx_s[:, :], idx_d[:, :]).then_inc(idx_sem, 16)
            if G:
                sync.dma_start(w_s[:, :], w_d[:, :]).then_inc(idx_sem, 16)
                sync.dma_start(cnts_s[:, :], cnts_d[:, :]).then_inc(idx_sem, 16)
            for ch in range(0, N_CHUNKS, 2):
                writeback(sync, ch)

        @block.scalar
        def _(scalar):
            for ch in range(1, N_CHUNKS, 2):
                writeback(scalar, ch)

        @block.gpsimd
        def _(gp):
            gp.load_library(mlp)
            gp.wait_ge(idx_sem, 16 * n_idx_dmas)

            def main_gather(ch):
                cs = CHUNK_SIZES[ch]
                # single_packet batches each engine's descriptors into large
                # packets: the engine round-robins across queues at packet
                # granularity, so without it the 4KB gather packets lose ~4:1
                # against the 16KB writeback packets and the late chunks crawl
                gp.dma_gather(mbuf[:, MOFF[ch]:MOFF[ch + 1], :], table[:, :],
                              idx_s[:, CPC_OFF[ch]:CPC_OFF[ch + 1]],
                              cs, cs, D,
                              single_packet=True).then_inc(g_sems[ch], 16)

            main_gather(0)
            main_gather(1)
            # the small cross-half gathers go early so the matmul chains that
            # need them can close mid-stream
            for i in range(n_x):
                gp.dma_gather(xbuf[:, i:i + 1, :], table[:, :],
                              idx_s[:, XOFS + i * 8:XOFS + (i + 1) * 8],
                              128, 128, D,
                              single_packet=False).then_inc(x_sems[i], 16)
            for ch in range(2, N_CHUNKS):
                main_gather(ch)
            for g in range(G):
                for c in range(wait_chunks[g]):         # target rows written
                    gp.wait_ge(w_sems[c], 16)
                gp.wait_ge(d_sem, g + 1)                # delta ready
                gp.reg_load(nreg, cnts_s[0:1, g:g + 1])
                gp.dma_scatter_add(out_d[:, :], delta[:, g:g + 1, :],
                                   idx_s[:, SOFS + g * 8:SOFS + (g + 1) * 8],
                                   128, nreg, D,
                                   single_packet=False).then_inc(s_sem, 16)
            if G:
                gp.wait_ge(s_sem, 16 * G)

        @block.tensor
        def _(te):
            if not G:
                return
            te.wait_ge(idx_sem, 16 * n_idx_dmas)
            waited = set()

            def mwait(sem, v, key):
                if key not in waited:
                    te.wait_ge(sem, v)
                    waited.add(key)

            for g in range(G):
                if g >= 2:                              # PSUM banks reused
                    te.wait_ge(d_sem, g - 1)
                base, pg = wbases[g], ps[g % 2]
                n_mm = len(ks[g]) + (1 if has_x[g] else 0)
                for ki, k in enumerate(ks[g]):
                    mwait(g_sems[_chunk_of_slot(k)], 16, ("g", _chunk_of_slot(k)))
                    for f in range(0, D, FSTEP):
                        ins = te.matmul(pg[:, f:f + FSTEP],
                                        w_s[:, base + ki * 128:base + (ki + 1) * 128],
                                        mbuf[:, k, f:f + FSTEP],
                                        start=(ki == 0), stop=(ki == n_mm - 1))
                        if ki == n_mm - 1:
                            ins.then_inc(mm_sem, 1)
                if has_x[g]:
                    xi = gx[g]
                    mwait(x_sems[xi], 16, ("x", xi))
                    xbase = base + len(ks[g]) * 128
                    for f in range(0, D, FSTEP):
                        te.matmul(pg[:, f:f + FSTEP],
                                  w_s[:, xbase:xbase + 128],
                                  xbuf[:, xi, f:f + FSTEP],
                                  start=(len(ks[g]) == 0),
                                  stop=True).then_inc(mm_sem, 1)

        @block.vector
        def _(v):
            if not G:
                return
            # drain each group's PSUM accumulator to a bf16 delta tile
            for g in range(G):
                v.wait_ge(mm_sem, (D // FSTEP) * (g + 1))
                v.tensor_scalar_mul(delta[:, g, :], ps[g % 2][:, :], 1.0
                                    ).then_inc(d_sem, 1)

    nc.compile()
    return nc


_CACHE = {}
_LAST_RESULT = None


def kernel(x, emb_table):
    global _LAST_RESULT
    from concourse.bass_utils import run_bass_kernel_spmd

    x_np = np.asarray(x)
    emb_np = np.asarray(emb_table, dtype=np.float32)
    uniq, cores, meta = _prepare(x_np)
    table_bf = np.ascontiguousarray(emb_np[uniq].astype(BF16))

    key = (meta["NV"], meta["G"], tuple(tuple(k) for k in meta["ks"]),
           tuple(meta["has_x"]), tuple(meta["wait_chunks"]))
    if key not in _CACHE:
        _CACHE[key] = _build_program(
            meta["NV"], meta["G"], meta["ks"], meta["has_x"],
            meta["wait_chunks"], meta.get("wbases"), meta.get("wcols"),
            meta.get("gx"), meta.get("n_x", 0))
    nc = _CACHE[key]

    in_maps = []
    for co in cores:
        m = {"table": table_bf, "idx": co["idx"]}
        if meta["G"]:
            m["W"] = co["W"]
            m["cnts"] = co["cnts"]
        in_maps.append(m)

    res = run_bass_kernel_spmd(nc, in_maps, core_ids=list(range(N_CORES)))
    _LAST_RESULT = res
    full = np.empty((B, S, D), dtype=np.float32)
    for c in range(N_CORES):
        b, h = c // 2, c % 2
        full[b, h * RPC:(h + 1) * RPC, :] = res.results[c]["out"].astype(np.float32)
    return full
